# revision 53
# baseline (speedup 1.0000x reference)
"""Trainium2 Bass kernel for nn_DTransformer (sparse attention w/ distance decay).

Sharding: data-parallel over batch (bs=8 -> 8 cores, one batch element per
core, weights replicated, no collectives).  Per core the full 3-layer network
runs on-chip.  All matmul operands are bf16 (PSUM accumulation stays f32);
1/sqrt(dk) and the LayerNorm affine of h1/h2/hh are folded into weights on
the host; the causal mask is fused into the PSUM->SBUF score copy; layers 1
and 2 are interleaved at q-tile granularity so the PE-heavy and ACT/DVE-heavy
stages of the two independent layers overlap.
"""

import os
import sys
import contextlib

for _p in ("/opt/trn_rl_repo", "/root/.axon_site/_ro/trn_rl_repo"):
    if os.path.isdir(_p) and _p not in sys.path:
        sys.path.insert(0, _p)

import numpy as np
import ml_dtypes

import concourse.bass as bass
import concourse.mybir as mybir
import concourse.tile as tile
from concourse import bacc

F32 = mybir.dt.float32
F16 = mybir.dt.float16
BF16 = mybir.dt.bfloat16
AF = mybir.ActivationFunctionType
OP = mybir.AluOpType

D = 256
H = 8
HG = 4            # heads per group
NG = H // HG
DK = 32
SEQ = 1024
BS = 8
NQT = SEQ // 128
ISQ = float(1.0 / np.sqrt(np.float32(DK)))
MASKV = -60000.0  # added to already-ISQ-scaled scores; exp() underflows to 0
EPS = 1e-5

bf16 = ml_dtypes.bfloat16
KEEP0 = frozenset({0})


def _opt(ap):
    return ap.opt(keep_dims=KEEP0)


def _rev(ap):
    """Reverse the innermost free dim of an AP (squeeze count-1 dims)."""
    pairs = [list(x) for x in ap.ap]
    keep = [pairs[0]] + [x for x in pairs[1:] if x[1] != 1]
    assert len(keep) == 2, f"need 2D-able ap, got {ap.ap}"
    (ps, pc), (fs, fc) = keep
    return bass.AP(tensor=ap.tensor, offset=ap.offset + fs * (fc - 1),
                   ap=[[ps, pc], [-fs, fc]])


def _bc(ap, n):
    """Append a broadcast innermost free dim of size n."""
    pairs = [list(x) for x in ap.ap]
    return bass.AP(tensor=ap.tensor, offset=ap.offset, ap=pairs + [[0, n]])


def _bcmid(ap, n):
    """Insert a broadcast middle free dim of size n after the partition dim."""
    pairs = [list(x) for x in ap.ap]
    return bass.AP(tensor=ap.tensor, offset=ap.offset,
                   ap=[pairs[0], [0, n]] + pairs[1:])


# ---------------------------------------------------------------- host prep

def host_prep(inputs):
    g = {k: np.asarray(v) for k, v in inputs.items()}

    def f32(x):
        return np.ascontiguousarray(np.asarray(x, dtype=np.float32))

    def b16(x):
        return np.ascontiguousarray(np.asarray(x, dtype=np.float32).astype(bf16))

    drv = {}
    # L1/L2 use the same projection for Q and K (Wk=Wq), so each side
    # carries sqrt(1/sqrt(dk)); their product is the 1/sqrt(dk) scale.
    SISQ = float(np.sqrt(ISQ))
    Wq1 = f32(g["Wq1"]) * SISQ
    Wq2 = f32(g["Wq2"]) * SISQ
    # LN affine of h1 folds into Wk3/bk3; of h2 into Wv3/bv3; of hh into Wlv.
    lng1, lnb1 = f32(g["lng1"]), f32(g["lnb1"])
    lng2, lnb2 = f32(g["lng2"]), f32(g["lnb2"])
    lng3, lnb3 = f32(g["lng3"]), f32(g["lnb3"])
    Wk3 = f32(g["Wk3"]) * lng1[None, :]
    bk3 = f32(g["bk3"]) + f32(g["Wk3"]) @ lnb1
    Wv3 = f32(g["Wv3"]) * lng2[None, :]
    bv3 = f32(g["bv3"]) + f32(g["Wv3"]) @ lnb2

    def f16(x):
        return np.ascontiguousarray(
            np.asarray(x, dtype=np.float32).astype(np.float16))

    WT = {
        "WT_q1": Wq1.T, "WT_v1": f32(g["Wv1"]).T, "WT_o1": f32(g["Wo1"]).T,
        "WT_q2": Wq2.T, "WT_v2": f32(g["Wv2"]).T, "WT_o2": f32(g["Wo2"]).T,
        "WT_k3": Wk3.T, "WT_v3": Wv3.T, "WT_o3": f32(g["Wo3"]).T,
    }
    for k, v in WT.items():
        drv[k] = f16(v)                                   # [din, dout] f16
    for nm, arr in (("bq1", f32(g["bq1"]) * SISQ),
                    ("bq2", f32(g["bq2"]) * SISQ), ("bk3", bk3)):
        drv[nm + "_c"] = f32(arr.reshape(2, 128).T)       # [128, 2] column
    drv["bv1_r"] = f32(g["bv1"]).reshape(1, D)
    drv["bv2_r"] = f32(g["bv2"]).reshape(1, D)
    drv["bv3_r"] = f32(bv3).reshape(1, D)
    for nm in ("bo1", "bo2", "bo3"):
        drv[nm + "_r"] = b16(f32(g[nm]).reshape(1, D))
    for i in (1, 2, 3):
        gam = -np.logaddexp(0.0, f32(g[f"g{i}"]).reshape(H))
        drv[f"gam2_{i}"] = f32((gam * gam).reshape(1, H))

    know = f32(g["know"]).reshape(D)
    q3 = know @ f32(g["Wq3"]).T + f32(g["bq3"])
    q3blk = np.zeros((D, H), np.float32)
    for h in range(H):
        q3blk[h * DK:(h + 1) * DK, h] = q3[h * DK:(h + 1) * DK] * ISQ
    drv["q3blk"] = f16(q3blk)
    drv["know_r"] = f32(know.reshape(1, D))
    kk = know.reshape(H, DK) @ f32(g["Wlk"]).T + f32(g["blk"])
    kk = 1.0 / (1.0 + np.exp(-kk))
    drv["kkT"] = f16(kk.T)                                # [256, 8]
    # per-head Wlv with lng3 folded: rows 32h..32h+31 = (Wlv*diag(lng3_h)).T
    Wlv = f32(g["Wlv"])                                   # [256, 32]
    blv = f32(g["blv"])
    wlvt8 = np.zeros((D, D), np.float32)
    blv8 = np.zeros((H, D), np.float32)
    for h in range(H):
        wlvt8[h * DK:(h + 1) * DK, :] = (Wlv * lng3[None, h * DK:(h + 1) * DK]).T
        blv8[h] = blv + Wlv @ lnb3[h * DK:(h + 1) * DK]
    drv["WlvT8"] = f16(wlvt8)                             # [256, 256]
    drv["blv8r"] = f16(blv8.reshape(1, H * D))            # [1, 2048]

    p = np.arange(128)[:, None]
    j = np.arange(128)[None, :]
    pos = np.concatenate(
        [np.abs((7 - ob) * 128 + p - j).astype(np.float32) for ob in range(8)],
        axis=1)
    drv["REVPOS"] = np.ascontiguousarray(pos.astype(bf16))
    drv["M0"] = f32(np.where(j <= p, 0.0, MASKV))         # inclusive causal
    drv["M3"] = np.ascontiguousarray(
        np.where(j < p, 0.0, MASKV).astype(bf16))         # strict causal
    drv["IDF"] = f32(np.eye(128))
    drv["IDB"] = np.ascontiguousarray(np.eye(128).astype(bf16))
    drv["IDH"] = f16(np.eye(128))
    return drv


# ---------------------------------------------------------------- builder

class KB:
    def __init__(self, nc, tc, ctx):
        self.nc, self.tc, self.ctx = nc, tc, ctx

    def pst(self, shape):
        """Shared small PSUM scratch (single tag, <=512 f32 per partition)."""
        return self.pps.tile(shape, F32, tag="ps", name="ps")

    def load_consts(self, dd):
        nc = self.nc
        pool = self.ctx.enter_context(self.tc.tile_pool(name="consts", bufs=1))
        sb = {}
        for nm in ("WT_q1", "WT_v1", "WT_o1", "WT_q2", "WT_v2", "WT_o2",
                   "WT_k3", "WT_v3", "WT_o3", "WlvT8"):
            t = pool.tile([128, 2, D], F16, tag=nm)
            nc.sync.dma_start(
                out=t[:],
                in_=dd[nm][:].rearrange("(a p) d -> p a d", p=128))
            sb[nm] = t
        for nm in ("q3blk", "kkT"):
            t = pool.tile([128, 2, H], F16, tag=nm)
            nc.sync.dma_start(
                out=t[:], in_=dd[nm][:].rearrange("(a p) h -> p a h", p=128))
            sb[nm] = t
        for nm in ("bq1_c", "bq2_c", "bk3_c", "REVPOS", "M0", "M3",
                   "IDF", "IDB", "IDH", "bo1_r", "bo2_r", "bo3_r", "blv8r"):
            src = dd[nm]
            t = pool.tile(list(src.shape), src.dtype, tag=nm)
            nc.sync.dma_start(out=t[:], in_=src[:])
            sb[nm] = t
        for nm in ("bv1_r", "bv2_r", "bv3_r", "know_r", "gam2_1", "gam2_2",
                   "gam2_3"):
            src = dd[nm]
            n = src.shape[1]
            t = pool.tile([128, n], F32, tag=nm)
            nc.sync.dma_start(
                out=t[:],
                in_=bass.AP(tensor=src, offset=0, ap=[[0, 128], [1, n]]))
            sb[nm] = t
        t = pool.tile([128, H * D], F16, tag="blv8rep")
        nc.sync.dma_start(
            out=t[:],
            in_=bass.AP(tensor=dd["blv8r"], offset=0,
                        ap=[[0, 128], [1, H * D]]))
        sb["blv8rep"] = t
        ones = pool.tile([1, 128], BF16, tag="ones")
        nc.vector.memset(ones[:], 1.0)
        sb["ones"] = ones
        onesh = pool.tile([128, 128], F16, tag="onesh")
        nc.vector.memset(onesh[:], 1.0)
        sb["ONESH"] = onesh
        onesf = pool.tile([1, 128], F32, tag="onesf")
        nc.vector.memset(onesf[:], 1.0)
        sb["onesf"] = onesf
        epst = pool.tile([128, 1], F32, tag="eps")
        nc.vector.memset(epst[:], EPS)
        sb["eps"] = epst
        self.sb = sb
        # pre-touch identities on PE so later transposes carry a single
        # DMA-queue wait (walrus allows only one sync wait on LDWEIGHTS)
        junk = pool.tile([128, 2], F32, tag="junk")
        wf = self.pps.tile([128, 128], F32, tag="ps", name="warmf")
        nc.tensor.transpose(wf[:], sb["IDF"][:], sb["IDF"][:])
        nc.scalar.copy(out=junk[:, 0:1], in_=wf[:, 0:1])
        wb = self.pps.tile([128, 128], BF16, tag="ps", name="warmb")
        nc.tensor.transpose(wb[:], sb["IDB"][:], sb["IDB"][:])
        nc.scalar.copy(out=junk[:, 1:2], in_=wb[:, 0:1])

    def hslice(self, T, h, cols):
        """Head-rows slice of a [128, 2, SEQ] transposed tensor: [32, len]."""
        return _opt(T[(h % 4) * DK:(h % 4 + 1) * DK, h // 4, cols])

    def load_nat(self, dram, pool, tag):
        tiles = []
        for st in range(NQT):
            t = pool.tile([128, D], F16, tag=f"{tag}{st}")
            self.nc.sync.dma_start(out=t[:],
                                   in_=dram[st * 128:(st + 1) * 128, :])
            tiles.append(t)
        return tiles

    def transpose_nat(self, x_tiles, pool, tag, dt=F16):
        """natural f16 [8][128,256] (tiles or APs) -> [128, 2, 1024] f16."""
        nc = self.nc
        xT = pool.tile([128, 2, SEQ], dt, tag=tag)
        for st in range(NQT):
            ps = self.pps.tile([128, 2, 128], F16, tag="ps", name="tp")
            for dh in range(2):
                nc.tensor.transpose(_opt(ps[:, dh, :]),
                                    _opt(x_tiles[st][:, dh * 128:(dh + 1) * 128]),
                                    self.sb["IDH"][:])
            nc.scalar.copy(out=_opt(xT[:, :, st * 128:(st + 1) * 128]),
                           in_=ps[:])
        return xT

    def proj_T(self, xT, wname, bname, pool, tag):
        """out[do, s] = W @ x.T + b : [128, 2, 1024] bf16."""
        nc = self.nc
        W = self.sb[wname]
        out = pool.tile([128, 2, SEQ], F16, tag=tag)
        for dh in range(2):
            for sc in range(2):
                ps = self.pst([128, 512])
                for ih in range(2):
                    nc.tensor.matmul(
                        ps[:], _opt(W[:, ih, dh * 128:(dh + 1) * 128]),
                        _opt(xT[:, ih, sc * 512:(sc + 1) * 512]),
                        start=(ih == 0), stop=(ih == 1))
                nc.scalar.activation(
                    out=_opt(out[:, dh, sc * 512:(sc + 1) * 512]), in_=ps[:],
                    func=AF.Identity, bias=self.sb[bname][:, dh:dh + 1],
                    scale=1.0)
        return out

    def proj_V(self, xT, wname, bname, pool, tag):
        """V natural with ones column: [8][128, H, 33] bf16."""
        nc = self.nc
        W = self.sb[wname]
        bias = self.sb[bname]
        tiles = []
        for st in range(NQT):
            ps = self.pst([128, D])
            for ih in range(2):
                nc.tensor.matmul(ps[:],
                                 _opt(xT[:, ih, st * 128:(st + 1) * 128]),
                                 _opt(W[:, ih, :]),
                                 start=(ih == 0), stop=(ih == 1))
            v = pool.tile([128, H, DK + 1], BF16, tag=f"{tag}{st}")
            nc.vector.tensor_tensor(
                out=v[:, :, 0:DK],
                in0=ps[:].rearrange("p (h d) -> p h d", h=H),
                in1=bias[:].rearrange("p (h d) -> p h d", h=H), op=OP.add)
            nc.vector.memset(_opt(v[:, :, DK:DK + 1]), 1.0)
            tiles.append(v)
        return tiles

    def make_attn_pools(self, lay, actx, attn_pool, psum, bufs=1, zs_bufs=1):
        tc = self.tc
        P = {"attn": [], "attn_pool": attn_pool}
        P["zq"], P["wtq"], P["oq"] = psum
        P["zsp"] = actx.enter_context(tc.tile_pool(name=f"zsp{lay}",
                                                   bufs=zs_bufs))
        P["ep"] = actx.enter_context(tc.tile_pool(name=f"ep{lay}", bufs=bufs))
        P["sp"] = actx.enter_context(tc.tile_pool(name=f"sp{lay}", bufs=bufs))
        P["sml"] = actx.enter_context(
            tc.tile_pool(name=f"sml{lay}", bufs=2 if lay == 3 else 4))
        P["wts"] = actx.enter_context(
            tc.tile_pool(name=f"wts{lay}", bufs=2 if lay == 3 else 3))
        return P

    # ------------------------------------------------ attention q-tile stages
    def attn_stages(self, lay, qt, QT, V, P, c3g=None, ncg=None,
                    cum3pad=None):
        """Return a list of 10 stage closures for one attention q-tile.
        Interleaving the stage lists of two independent streams keeps every
        engine fed and pairs same-function ACT calls (fewer table loads)."""
        nc, sb = self.nc, self.sb
        strict = (lay == 3)
        Kt = (qt + 1) * 128
        rp_ap = _opt(sb["REVPOS"][:, (7 - qt) * 128:(7 - qt) * 128 + Kt])
        at = P["attn_pool"].tile([128, H, DK], F16, tag=f"attn{lay}_{qt}")
        ot = P["oq"].tile([128, H, DK + 1], F32, tag="o", name="o")
        zs = P["zsp"].tile([128, H, Kt], F16, tag="zs", name="zs")
        e = P["ep"].tile([128, H, Kt], BF16, tag="e", name="e")
        S = P["sp"].tile([128, H, Kt], BF16, tag="S", name="S")
        st_ = {}

        if not strict:
            def s0():  # scores -> PSUM -> zs (+fused causal mask)
                for h in range(H):
                    lhs = self.hslice(QT, h, slice(qt * 128, qt * 128 + 128))
                    tp = ((h % 4) * DK, 0)
                    nchunk = (Kt + 511) // 512
                    for ci in range(nchunk):
                        kc = ci * 512
                        cl = min(512, Kt - kc)
                        z = P["zq"].tile([128, cl], F32, tag="z", name="z")
                        nc.tensor.matmul(
                            z[:], lhs,
                            self.hslice(QT, h, slice(kc, kc + cl)),
                            start=True, stop=True,
                            tile_position=tp, skip_group_check=True)
                        if qt == 0:
                            nc.vector.scalar_tensor_tensor(
                                out=_opt(zs[:, h, :]), in0=z[:], scalar=1.0,
                                in1=sb["M0"][:], op0=OP.mult, op1=OP.add)
                        elif h % 2 == 0:
                            nc.vector.tensor_copy(
                                out=_opt(zs[:, h, kc:kc + cl]), in_=z[:])
                        else:
                            nc.scalar.copy(
                                out=_opt(zs[:, h, kc:kc + cl]), in_=z[:])
                if qt > 0:
                    dg = _opt(zs[:, :, Kt - 128:Kt])
                    nc.vector.scalar_tensor_tensor(
                        out=dg, in0=dg, scalar=1.0,
                        in1=_bcmid(sb["M0"][:], H), op0=OP.mult, op1=OP.add)

            def s1():  # e = exp(zs)
                for hg in range(NG):
                    hs = hg * HG
                    nc.scalar.activation(out=_opt(e[:, hs:hs + HG, :]),
                                         in_=_opt(zs[:, hs:hs + HG, :]),
                                         func=AF.Exp)

            def s2():  # reversed scans -> S; E; rEg
                for h in range(H):
                    nc.vector.tensor_tensor_scan(
                        out=_rev(S[:, h, 0:Kt - 1]),
                        data0=_rev(e[:, h, 1:Kt]), data1=_rev(e[:, h, 1:Kt]),
                        initial=0.0, op0=OP.add, op1=OP.bypass)
                    nc.vector.memset(_opt(S[:, h, Kt - 1:Kt]), 0.0)
                E = P["sml"].tile([128, H], F32, tag="E", name="E")
                rEg = P["sml"].tile([128, H], F32, tag="rEg", name="rEg")
                nc.vector.tensor_tensor(out=E[:], in0=_opt(S[:, :, 0:1]),
                                        in1=_opt(e[:, :, 0:1]), op=OP.add)
                nc.vector.reciprocal(out=rEg[:], in_=E[:])
                nc.vector.tensor_tensor(out=rEg[:], in0=rEg[:],
                                        in1=sb[f"gam2_{lay}"][:], op=OP.mult)
                st_["rEg"] = rEg

            def s3():  # u = (S * rEg_h) * pos  (in place on S)
                rEg = st_["rEg"]
                for h in range(H):
                    nc.vector.scalar_tensor_tensor(
                        out=_opt(S[:, h, :]), in0=_opt(S[:, h, :]),
                        scalar=rEg[:, h:h + 1], in1=rp_ap,
                        op0=OP.mult, op1=OP.mult)

            def s4():  # d = sqrt(u)
                for hg in range(NG):
                    hs = hg * HG
                    nc.scalar.activation(out=_opt(S[:, hs:hs + HG, :]),
                                         in_=_opt(S[:, hs:hs + HG, :]),
                                         func=AF.Sqrt)

            def s5():  # eff = exp(-d)
                for hg in range(NG):
                    hs = hg * HG
                    nc.scalar.activation(out=_opt(S[:, hs:hs + HG, :]),
                                         in_=_opt(S[:, hs:hs + HG, :]),
                                         func=AF.Exp, scale=-1.0)

            def s6():  # t = eff * zs  (in place on zs; 1e-5 clamp dropped)
                for hg in range(NG):
                    hs = hg * HG
                    nc.vector.tensor_tensor(out=_opt(zs[:, hs:hs + HG, :]),
                                            in0=_opt(S[:, hs:hs + HG, :]),
                                            in1=_opt(zs[:, hs:hs + HG, :]),
                                            op=OP.mult)

            def s7():  # w = exp(t)  (into e's tile)
                for hg in range(NG):
                    hs = hg * HG
                    nc.scalar.activation(out=_opt(e[:, hs:hs + HG, :]),
                                         in_=_opt(zs[:, hs:hs + HG, :]),
                                         func=AF.Exp)
        else:
            def s0():  # E3/rE3; u = (E3 - cum3) * pos (into S); diag clamp
                e3ps = self.pst([128, 8])
                nc.tensor.transpose(
                    e3ps[:], _opt(cum3pad[:, qt * 128:qt * 128 + 128]),
                    _opt(sb["IDF"][0:8, 0:8]))
                E3 = P["sml"].tile([128, H], F32, tag="E3", name="E3")
                nc.vector.tensor_scalar_max(out=E3[:], in0=e3ps[:],
                                            scalar1=1e-30)
                rE3 = P["sml"].tile([128, H], F32, tag="rE3", name="rE3")
                nc.vector.reciprocal(out=rE3[:], in_=E3[:])
                nc.vector.tensor_tensor(out=rE3[:], in0=rE3[:],
                                        in1=sb["gam2_3"][:], op=OP.mult)
                st_["rE3"] = rE3
                for h in range(H):
                    nc.vector.scalar_tensor_tensor(
                        out=_opt(S[:, h, :]), in0=_opt(ncg[:, h, 0:Kt]),
                        scalar=_opt(E3[:, h:h + 1]), in1=rp_ap,
                        op0=OP.add, op1=OP.mult)
                nc.vector.tensor_scalar_max(
                    out=_opt(S[:, :, Kt - 128:Kt]),
                    in0=_opt(S[:, :, Kt - 128:Kt]), scalar1=0.0)

            def s1():
                pass

            def s2():
                pass

            def s3():
                pass

            def s4():  # d = sqrt(u * gamma^2/E3) per head (scale AP)
                rE3 = st_["rE3"]
                for h in range(H):
                    nc.scalar.activation(
                        out=_opt(S[:, h, :]), in_=_opt(S[:, h, :]),
                        func=AF.Sqrt, scale=_opt(rE3[:, h:h + 1]))

            def s5():  # eff = exp(-d)
                for hg in range(NG):
                    hs = hg * HG
                    nc.scalar.activation(out=_opt(S[:, hs:hs + HG, :]),
                                         in_=_opt(S[:, hs:hs + HG, :]),
                                         func=AF.Exp, scale=-1.0)

            def s6():  # t = eff * c3 (masked diag) into zs
                if qt > 0:
                    nc.vector.tensor_tensor(
                        out=_opt(zs[:, :, 0:Kt - 128]),
                        in0=_opt(S[:, :, 0:Kt - 128]),
                        in1=_opt(c3g[:, :, 0:Kt - 128]), op=OP.mult)
                c3m = P["sml"].tile([128, H, 128], BF16, tag="c3m",
                                    name="c3m")
                nc.vector.tensor_tensor(
                    out=c3m[:], in0=_opt(c3g[:, :, Kt - 128:Kt]),
                    in1=_bcmid(sb["M3"][:], H), op=OP.add)
                nc.vector.tensor_tensor(
                    out=_opt(zs[:, :, Kt - 128:Kt]),
                    in0=_opt(S[:, :, Kt - 128:Kt]), in1=c3m[:], op=OP.mult)

            def s7():  # w = exp(t)  (into e's tile)
                for hg in range(NG):
                    hs = hg * HG
                    nc.scalar.activation(out=_opt(e[:, hs:hs + HG, :]),
                                         in_=_opt(zs[:, hs:hs + HG, :]),
                                         func=AF.Exp)

        def s8():  # transpose w via PE; A @ [V | 1] accumulation
            for h in range(H):
                nblk = qt + 1
                for g0 in range(0, nblk, 4):
                    gl = min(4, nblk - g0)
                    wt_ps = P["wtq"].tile([128, 512], BF16, tag="wt",
                                          name="wt")
                    for j in range(gl):
                        kb = g0 + j
                        nc.tensor.transpose(
                            _opt(wt_ps[:, j * 128:(j + 1) * 128]),
                            _opt(e[:, h, kb * 128:(kb + 1) * 128]),
                            sb["IDB"][:])
                    wt_sb = P["wts"].tile([128, 512], BF16, tag="wts",
                                          name="wts")
                    if (h + g0 // 4) % 2 == 0:
                        nc.vector.tensor_copy(
                            out=_opt(wt_sb[:, 0:gl * 128]),
                            in_=_opt(wt_ps[:, 0:gl * 128]))
                    else:
                        nc.scalar.copy(
                            out=_opt(wt_sb[:, 0:gl * 128]),
                            in_=_opt(wt_ps[:, 0:gl * 128]))
                    for j in range(gl):
                        kb = g0 + j
                        nc.tensor.matmul(
                            _opt(ot[:, h, :]),
                            _opt(wt_sb[:, j * 128:(j + 1) * 128]),
                            _opt(V[kb][:, h, :]),
                            start=(kb == 0), stop=(kb == qt),
                            skip_group_check=True)

        def s9():  # normalize attn = o / W
            Wg = P["sml"].tile([128, H], F32, tag="Wg", name="Wg")
            rW = P["sml"].tile([128, H], F32, tag="rW", name="rW")
            nc.vector.tensor_scalar_max(
                out=Wg[:], in0=_opt(ot[:, :, DK:DK + 1]), scalar1=1e-30)
            nc.vector.reciprocal(out=rW[:], in_=Wg[:])
            nc.vector.tensor_tensor(
                out=at[:], in0=_opt(ot[:, :, 0:DK]), in1=_bc(rW[:], DK),
                op=OP.mult)
            P["attn"].append(at)

        return [s0, s1, s2, s3, s4, s5, s6, s7, s8, s9]

    # ------------------------------------------------ out proj + LN
    def out_ln(self, lay, attn, res_tiles, hpool, spool, tpool):
        """Out-projection + residual + LN (affine folded downstream).
        Batches the LN sqrt calls to minimize ACT table flips."""
        nc, sb = self.nc, self.sb
        attnT = self.transpose_nat(
            [a[:].rearrange("p h d -> p (h d)") for a in attn],
            tpool, tag=f"attnT{lay}")
        W = sb[f"WT_o{lay}"]
        xs, mvs = [], []
        for st in range(NQT):
            ps = self.pst([128, D])
            for ih in range(2):
                nc.tensor.matmul(ps[:],
                                 _opt(attnT[:, ih, st * 128:(st + 1) * 128]),
                                 _opt(W[:, ih, :]), start=(ih == 0), stop=False)
            nc.tensor.matmul(ps[:], sb["ones"][:], sb[f"bo{lay}_r"][:],
                             start=False, stop=True)
            res = res_tiles[st] if isinstance(res_tiles, list) else res_tiles
            x = tpool.tile([128, D], F16, tag=f"lnx{lay}_{st}", name="lnx")
            nc.vector.tensor_tensor(out=x[:], in0=ps[:], in1=res[:], op=OP.add)
            stats = spool.tile([128, 6], F32, tag="bnst", name="bnst")
            mv = tpool.tile([128, 2], F32, tag=f"bnmv{lay}_{st}", name="bnmv")
            nc.vector.bn_stats(out=stats[:], in_=x[:])
            nc.vector.bn_aggr(out=mv[:], in_=stats[:])
            xs.append(x)
            mvs.append(mv)
        sds = spool.tile([128, NQT], F32, tag="sds", name="sds")
        for st in range(NQT):
            nc.scalar.activation(out=_opt(sds[:, st:st + 1]),
                                 in_=_opt(mvs[st][:, 1:2]), func=AF.Sqrt,
                                 bias=sb["eps"][:], scale=1.0)
        rstds = spool.tile([128, NQT], F32, tag="rstds", name="rstds")
        nc.vector.reciprocal(out=rstds[:], in_=sds[:])
        out_tiles = []
        for st in range(NQT):
            ho = hpool.tile([128, D], F16, tag=f"h{lay}_{st}")
            nc.vector.tensor_scalar(
                out=ho[:], in0=xs[st][:], scalar1=_opt(mvs[st][:, 0:1]),
                scalar2=_opt(rstds[:, st:st + 1]),
                op0=OP.subtract, op1=OP.mult)
            out_tiles.append(ho)
        return out_tiles

    # ------------------------------------------------ layer-3 prologue
    def l3_rows(self, h1T, mpool, lpool):
        nc, sb = self.nc, self.sb
        KT3 = self.proj_T(h1T, "WT_k3", "bk3_c", mpool, tag="KT3")
        c3 = mpool.tile([8, SEQ], BF16, tag="c3")
        c3f = mpool.tile([8, SEQ], F32, tag="c3f")
        for sc in range(2):
            ps = self.pst([8, 512])
            for ih in range(2):
                nc.tensor.matmul(ps[:], _opt(sb["q3blk"][:, ih, :]),
                                 _opt(KT3[:, ih, sc * 512:(sc + 1) * 512]),
                                 start=(ih == 0), stop=(ih == 1))
            nc.vector.tensor_copy(out=_opt(c3[:, sc * 512:(sc + 1) * 512]),
                                  in_=ps[:])
            nc.scalar.copy(out=_opt(c3f[:, sc * 512:(sc + 1) * 512]),
                           in_=ps[:])
        e3 = mpool.tile([8, SEQ], F32, tag="e3")
        nc.scalar.activation(out=e3[:], in_=c3f[:], func=AF.Exp)
        cum3pad = lpool.tile([8, SEQ + 128], F32, tag="cum3pad")
        nc.vector.memset(_opt(cum3pad[:, 0:1]), 0.0)
        nc.vector.tensor_tensor_scan(
            out=_opt(cum3pad[:, 1:SEQ + 1]), data0=e3[:], data1=e3[:],
            initial=0.0, op0=OP.add, op1=OP.bypass)
        nc.vector.memset(_opt(cum3pad[:, SEQ + 1:]), 0.0)
        # stage all 8 head-rows to partition 0 (two DMAs)
        stc = mpool.tile([1, 8, SEQ], BF16, tag="stc", name="stc")
        stn = mpool.tile([1, 8, SEQ], F32, tag="stn", name="stn")
        nc.sync.dma_start(out=stc[:], in_=c3[:])
        nc.sync.dma_start(out=stn[:], in_=cum3pad[:, 1:SEQ + 1])
        c3g = lpool.tile([128, H, SEQ], BF16, tag="c3g")
        ncg = lpool.tile([128, H, SEQ], F32, tag="ncg")
        for h in range(H):
            for sc in range(2):
                ps = self.pst([128, 512])
                nc.tensor.matmul(ps[:], sb["ones"][:],
                                 _opt(stc[:, h, sc * 512:(sc + 1) * 512]),
                                 start=True, stop=True)
                nc.scalar.copy(out=_opt(c3g[:, h, sc * 512:(sc + 1) * 512]),
                               in_=ps[:])
                ps2 = self.pst([128, 512])
                nc.tensor.matmul(ps2[:], sb["onesf"][:],
                                 _opt(stn[:, h, sc * 512:(sc + 1) * 512]),
                                 start=True, stop=True)
                nc.vector.tensor_scalar_mul(
                    out=_opt(ncg[:, h, sc * 512:(sc + 1) * 512]), in0=ps2[:],
                    scalar1=-1.0)
        return c3g, ncg, cum3pad

    # ------------------------------------------------ final mixture
    def final_alphas(self, xT_q, tpool, spool):
        """alpha = softmax(kk @ q_emb) -- depends only on x_q; emitted early."""
        nc, sb = self.nc, self.sb
        als = []
        for st in range(NQT):
            bps = self.pst([128, H])
            for ih in range(2):
                nc.tensor.matmul(bps[:],
                                 _opt(xT_q[:, ih, st * 128:(st + 1) * 128]),
                                 _opt(sb["kkT"][:, ih, :]),
                                 start=(ih == 0), stop=(ih == 1))
            nmax = spool.tile([128, 1], F32, tag="nmax", name="nmax")
            nc.vector.tensor_reduce(out=nmax[:], in_=bps[:],
                                    axis=mybir.AxisListType.X, op=OP.max,
                                    negate=True)
            au = spool.tile([128, H], F32, tag="au", name="au")
            sa = spool.tile([128, 1], F32, tag="sa", name="sa")
            nc.scalar.activation(out=au[:], in_=bps[:], func=AF.Exp,
                                 bias=nmax[:], scale=1.0, accum_out=sa[:])
            rsa = spool.tile([128, 1], F32, tag="rsa", name="rsa")
            nc.vector.reciprocal(out=rsa[:], in_=sa[:])
            al = tpool.tile([128, H], F32, tag=f"al{st}", name="al")
            nc.vector.tensor_scalar_mul(out=al[:], in0=au[:], scalar1=rsa[:])
            als.append(al)
        return als

    def final(self, hh, als, out_dram, spool, tpool, vq):
        nc, sb = self.nc, self.sb
        hhT = self.transpose_nat(hh, tpool, tag="hhT")
        for st in range(NQT):
            al = als[st]
            acc = spool.tile([128, D], F32, tag="facc", name="facc")
            for h in range(H):
                o = (h % 4) * DK
                vps = vq.tile([128, D], F32, tag="vps", name="vps")
                nc.tensor.matmul(
                    vps[:],
                    self.hslice(hhT, h, slice(st * 128, st * 128 + 128)),
                    _opt(sb["WlvT8"][o:o + DK, h // 4, :]),
                    start=True, stop=False,
                    tile_position=(o, 0), skip_group_check=True)
                nc.tensor.matmul(
                    vps[:], _opt(sb["ONESH"][o:o + 1, :]),
                    _opt(sb["blv8rep"][o:o + 1, h * D:(h + 1) * D]),
                    start=False, stop=True,
                    tile_position=(o, 0), skip_group_check=True)
                vsb = spool.tile([128, D], F16, tag="vsb", name="vsb")
                nc.scalar.activation(out=vsb[:], in_=vps[:], func=AF.Sigmoid)
                if h == 0:
                    nc.vector.tensor_scalar_mul(
                        out=acc[:], in0=vsb[:], scalar1=al[:, 0:1])
                else:
                    nc.vector.scalar_tensor_tensor(
                        out=acc[:], in0=vsb[:], scalar=al[:, h:h + 1],
                        in1=acc[:], op0=OP.mult, op1=OP.add)
            nc.sync.dma_start(out=out_dram[st * 128:(st + 1) * 128, :],
                              in_=acc[:])


def build(derived, debug=False, stop_after=None):
    nc = bacc.Bacc(None, target_bir_lowering=False)
    dd = {}
    for name, arr in derived.items():
        dt = {np.dtype(np.float32): F32, np.dtype(bf16): BF16,
              np.dtype(np.float16): F16}[np.dtype(arr.dtype)]
        dd[name] = nc.dram_tensor(name, list(arr.shape), dt,
                                  kind="ExternalInput")
    x_q = nc.dram_tensor("x_q", [SEQ, D], F16, kind="ExternalInput")
    x_s = nc.dram_tensor("x_s", [SEQ, D], F16, kind="ExternalInput")
    out = nc.dram_tensor("out", [SEQ, D], F32, kind="ExternalOutput")

    def dump(tiles, name):
        if not debug:
            return
        t = nc.dram_tensor(name, [SEQ, D], F16, kind="ExternalOutput")
        for st in range(NQT):
            ap = tiles[st][:]
            if len(ap.shape) == 3:
                ap = ap.rearrange("p h d -> p (h d)")
            nc.sync.dma_start(out=t[st * 128:(st + 1) * 128, :], in_=ap)

    with tile.TileContext(nc) as tc, contextlib.ExitStack() as ctx:
        kb = KB(nc, tc, ctx)
        kb.pps = ctx.enter_context(
            tc.tile_pool(name="pps", bufs=1, space="PSUM"))
        kb.load_consts(dd)
        sb = kb.sb
        glob = ctx.enter_context(tc.tile_pool(name="glob", bufs=1))
        hpool = ctx.enter_context(tc.tile_pool(name="hpool", bufs=1))

        h1 = h2 = None
        # ---------------- layers 1+2 interleaved ----------------
        with tc.tile_pool(name="rx", bufs=1) as rx, \
                tc.tile_pool(name="rs", bufs=2) as rs, \
                contextlib.ExitStack() as actx:
            xq_nat = kb.load_nat(x_q, rx, "xq")
            xs_nat = kb.load_nat(x_s, rx, "xs")
            xT_q = kb.transpose_nat(xq_nat, glob, tag="xTq")
            xT_s = kb.transpose_nat(xs_nat, rx, tag="xTs")
            QT1 = kb.proj_T(xT_q, "WT_q1", "bq1_c", rx, tag="QT1")
            QT2 = kb.proj_T(xT_s, "WT_q2", "bq2_c", rx, tag="QT2")
            V1 = kb.proj_V(xT_q, "WT_v1", "bv1_r", rx, tag="V1")
            V2 = kb.proj_V(xT_s, "WT_v2", "bv2_r", rx, tag="V2")
            als = kb.final_alphas(xT_q, glob, rs)
            if stop_after in (None, "l1", "l2", "pro", "l3"):
                zq = actx.enter_context(
                    tc.tile_pool(name="zq", bufs=3, space="PSUM"))
                wtq = actx.enter_context(
                    tc.tile_pool(name="wtq", bufs=2, space="PSUM"))
                oq = actx.enter_context(
                    tc.tile_pool(name="oq", bufs=2, space="PSUM"))
                psum = (zq, wtq, oq)
                P1 = kb.make_attn_pools(1, actx, rx, psum)
                P2 = kb.make_attn_pools(2, actx, rx, psum)
                for qt in range(NQT):
                    a = kb.attn_stages(1, qt, QT1, V1, P1)
                    b = kb.attn_stages(2, qt, QT2, V2, P2)
                    for f1, f2 in zip(a, b):
                        f1()
                        f2()
                attn1, attn2 = P1["attn"], P2["attn"]
                dump(attn1, "dbg_attn1")
                h1 = kb.out_ln(1, attn1, xq_nat, hpool, rs, rx)
                h2 = kb.out_ln(2, attn2, xs_nat, hpool, rs, rx)

        if stop_after in (None, "pro", "l3"):
            dump(h1, "dbg_h1")
            dump(h2, "dbg_h2")
        # ---------------- layer 3 prologue ----------------
        if stop_after in (None, "pro", "l3"):
            lpool = ctx.enter_context(tc.tile_pool(name="l3pool", bufs=1))
            with tc.tile_pool(name="l3tmp", bufs=1) as l3tmp:
                h1T = kb.transpose_nat(h1, l3tmp, tag="h1T")
                h2T = kb.transpose_nat(h2, l3tmp, tag="h2T")
                V3 = kb.proj_V(h2T, "WT_v3", "bv3_r", lpool, tag="V3")
                c3g, ncg, cum3pad = kb.l3_rows(h1T, l3tmp, lpool)
                if stop_after == "pro":
                    for st in range(NQT):
                        o32 = l3tmp.tile([128, D], F32, tag=f"o32_{st}")
                        nc.vector.tensor_copy(
                            out=o32[:].rearrange("p (h d) -> p h d", h=H),
                            in_=V3[st][:, :, 0:DK])
                        nc.sync.dma_start(out=out[st * 128:(st + 1) * 128, :],
                                          in_=o32[:])
        # ---------------- layer 3 + final ----------------
        if stop_after in (None, "l3"):
            with tc.tile_pool(name="r3s", bufs=2) as r3s, \
                    tc.tile_pool(name="r3", bufs=1) as r3, \
                    contextlib.ExitStack() as actx3:
                wtq3 = actx3.enter_context(
                    tc.tile_pool(name="wtq3", bufs=2, space="PSUM"))
                oq3 = actx3.enter_context(
                    tc.tile_pool(name="oq3", bufs=2, space="PSUM"))
                P3 = kb.make_attn_pools(3, actx3, r3, (None, wtq3, oq3),
                                        bufs=2, zs_bufs=1)
                for q0 in range(0, NQT, 2):
                    a = kb.attn_stages(3, q0, None, V3, P3, c3g=c3g,
                                       ncg=ncg, cum3pad=cum3pad)
                    b = kb.attn_stages(3, q0 + 1, None, V3, P3, c3g=c3g,
                                       ncg=ncg, cum3pad=cum3pad)
                    for fa, fb in zip(a, b):
                        fa()
                        fb()
                attn3 = P3["attn"]
                dump(attn3, "dbg_attn3")
                hh = kb.out_ln(3, attn3, sb["know_r"], r3, r3s, r3)
                dump(hh, "dbg_hh")
                if stop_after is None:
                    vq = actx3.enter_context(
                        tc.tile_pool(name="vq", bufs=2, space="PSUM"))
                    kb.final(hh, als, out, r3s, r3, vq)
    nc.compile()
    return nc


_CACHE = {}


def kernel(**inputs):
    drv = host_prep(inputs)
    if "nc" not in _CACHE:
        _CACHE["nc"] = build(drv)
    nc = _CACHE["nc"]
    q = np.asarray(inputs["q_emb"], np.float32).astype(np.float16)
    s = np.asarray(inputs["s_emb"], np.float32).astype(np.float16)
    in_maps = []
    for b in range(BS):
        m = dict(drv)
        m["x_q"] = np.ascontiguousarray(q[b])
        m["x_s"] = np.ascontiguousarray(s[b])
        in_maps.append(m)
    from concourse.bass_utils import run_bass_kernel_spmd
    res = run_bass_kernel_spmd(nc, in_maps, core_ids=list(range(BS)))
    out = np.stack([np.asarray(res.results[b]["out"]) for b in range(BS)],
                   axis=0)
    return out.astype(np.float32)


if __name__ == "__main__":
    print("kernel module loaded OK")


# revision 54
# speedup vs baseline: 1.0597x; 1.0597x over previous
"""Trainium2 Bass kernel for nn_DTransformer (sparse attention w/ distance decay).

Sharding: data-parallel over batch (bs=8 -> 8 cores, one batch element per
core, weights replicated, no collectives).  Per core the full 3-layer network
runs on-chip.  All matmul operands are bf16 (PSUM accumulation stays f32);
1/sqrt(dk) and the LayerNorm affine of h1/h2/hh are folded into weights on
the host; the causal mask is fused into the PSUM->SBUF score copy; layers 1
and 2 are interleaved at q-tile granularity so the PE-heavy and ACT/DVE-heavy
stages of the two independent layers overlap.
"""

import os
import sys
import contextlib

for _p in ("/opt/trn_rl_repo", "/root/.axon_site/_ro/trn_rl_repo"):
    if os.path.isdir(_p) and _p not in sys.path:
        sys.path.insert(0, _p)

import numpy as np
import ml_dtypes

import concourse.bass as bass
import concourse.mybir as mybir
import concourse.tile as tile
from concourse import bacc

F32 = mybir.dt.float32
F16 = mybir.dt.float16
BF16 = mybir.dt.bfloat16
AF = mybir.ActivationFunctionType
OP = mybir.AluOpType

D = 256
H = 8
HG = 4            # heads per group
NG = H // HG
DK = 32
SEQ = 1024
BS = 8
NQT = SEQ // 128
ISQ = float(1.0 / np.sqrt(np.float32(DK)))
MASKV = -60000.0  # added to already-ISQ-scaled scores; exp() underflows to 0
EPS = 1e-5

bf16 = ml_dtypes.bfloat16
KEEP0 = frozenset({0})


def _opt(ap):
    return ap.opt(keep_dims=KEEP0)


def _rev(ap):
    """Reverse the innermost free dim of an AP (squeeze count-1 dims)."""
    pairs = [list(x) for x in ap.ap]
    keep = [pairs[0]] + [x for x in pairs[1:] if x[1] != 1]
    assert len(keep) == 2, f"need 2D-able ap, got {ap.ap}"
    (ps, pc), (fs, fc) = keep
    return bass.AP(tensor=ap.tensor, offset=ap.offset + fs * (fc - 1),
                   ap=[[ps, pc], [-fs, fc]])


def _bc(ap, n):
    """Append a broadcast innermost free dim of size n."""
    pairs = [list(x) for x in ap.ap]
    return bass.AP(tensor=ap.tensor, offset=ap.offset, ap=pairs + [[0, n]])


def _bcmid(ap, n):
    """Insert a broadcast middle free dim of size n after the partition dim."""
    pairs = [list(x) for x in ap.ap]
    return bass.AP(tensor=ap.tensor, offset=ap.offset,
                   ap=[pairs[0], [0, n]] + pairs[1:])


# ---------------------------------------------------------------- host prep

def host_prep(inputs):
    g = {k: np.asarray(v) for k, v in inputs.items()}

    def f32(x):
        return np.ascontiguousarray(np.asarray(x, dtype=np.float32))

    def b16(x):
        return np.ascontiguousarray(np.asarray(x, dtype=np.float32).astype(bf16))

    drv = {}
    # L1/L2 use the same projection for Q and K (Wk=Wq), so each side
    # carries sqrt(1/sqrt(dk)); their product is the 1/sqrt(dk) scale.
    SISQ = float(np.sqrt(ISQ))
    Wq1 = f32(g["Wq1"]) * SISQ
    Wq2 = f32(g["Wq2"]) * SISQ
    # LN affine of h1 folds into Wk3/bk3; of h2 into Wv3/bv3; of hh into Wlv.
    lng1, lnb1 = f32(g["lng1"]), f32(g["lnb1"])
    lng2, lnb2 = f32(g["lng2"]), f32(g["lnb2"])
    lng3, lnb3 = f32(g["lng3"]), f32(g["lnb3"])
    Wk3 = f32(g["Wk3"]) * lng1[None, :]
    bk3 = f32(g["bk3"]) + f32(g["Wk3"]) @ lnb1
    Wv3 = f32(g["Wv3"]) * lng2[None, :]
    bv3 = f32(g["bv3"]) + f32(g["Wv3"]) @ lnb2

    def f16(x):
        return np.ascontiguousarray(
            np.asarray(x, dtype=np.float32).astype(np.float16))

    WT = {
        "WT_q1": Wq1.T, "WT_v1": f32(g["Wv1"]).T, "WT_o1": f32(g["Wo1"]).T,
        "WT_q2": Wq2.T, "WT_v2": f32(g["Wv2"]).T, "WT_o2": f32(g["Wo2"]).T,
        "WT_k3": Wk3.T, "WT_v3": Wv3.T, "WT_o3": f32(g["Wo3"]).T,
    }
    for k, v in WT.items():
        drv[k] = f16(v)                                   # [din, dout] f16
    for nm, arr in (("bq1", f32(g["bq1"]) * SISQ),
                    ("bq2", f32(g["bq2"]) * SISQ), ("bk3", bk3)):
        drv[nm + "_c"] = f32(arr.reshape(2, 128).T)       # [128, 2] column
    drv["bv1_r"] = f32(g["bv1"]).reshape(1, D)
    drv["bv2_r"] = f32(g["bv2"]).reshape(1, D)
    drv["bv3_r"] = f32(bv3).reshape(1, D)
    for nm in ("bo1", "bo2", "bo3"):
        drv[nm + "_r"] = b16(f32(g[nm]).reshape(1, D))
    for i in (1, 2, 3):
        gam = -np.logaddexp(0.0, f32(g[f"g{i}"]).reshape(H))
        drv[f"gam2_{i}"] = f32((gam * gam).reshape(1, H))

    know = f32(g["know"]).reshape(D)
    q3 = know @ f32(g["Wq3"]).T + f32(g["bq3"])
    q3blk = np.zeros((D, H), np.float32)
    for h in range(H):
        q3blk[h * DK:(h + 1) * DK, h] = q3[h * DK:(h + 1) * DK] * ISQ
    drv["q3blk"] = f16(q3blk)
    drv["know_r"] = f32(know.reshape(1, D))
    kk = know.reshape(H, DK) @ f32(g["Wlk"]).T + f32(g["blk"])
    kk = 1.0 / (1.0 + np.exp(-kk))
    drv["kkT"] = f16(kk.T)                                # [256, 8]
    # per-head Wlv with lng3 folded: rows 32h..32h+31 = (Wlv*diag(lng3_h)).T
    Wlv = f32(g["Wlv"])                                   # [256, 32]
    blv = f32(g["blv"])
    wlvt8 = np.zeros((D, D), np.float32)
    blv8 = np.zeros((H, D), np.float32)
    for h in range(H):
        wlvt8[h * DK:(h + 1) * DK, :] = (Wlv * lng3[None, h * DK:(h + 1) * DK]).T
        blv8[h] = blv + Wlv @ lnb3[h * DK:(h + 1) * DK]
    drv["WlvT8"] = f16(wlvt8)                             # [256, 256]
    drv["blv8r"] = f16(blv8.reshape(1, H * D))            # [1, 2048]

    p = np.arange(128)[:, None]
    j = np.arange(128)[None, :]
    pos = np.concatenate(
        [np.abs((7 - ob) * 128 + p - j).astype(np.float32) for ob in range(8)],
        axis=1)
    drv["REVPOS"] = np.ascontiguousarray(pos.astype(bf16))
    drv["M0"] = f32(np.where(j <= p, 0.0, MASKV))         # inclusive causal
    drv["M3"] = np.ascontiguousarray(
        np.where(j < p, 0.0, MASKV).astype(bf16))         # strict causal
    drv["IDF"] = f32(np.eye(128))
    drv["IDB"] = np.ascontiguousarray(np.eye(128).astype(bf16))
    drv["IDH"] = f16(np.eye(128))
    return drv


# ---------------------------------------------------------------- builder

class KB:
    def __init__(self, nc, tc, ctx):
        self.nc, self.tc, self.ctx = nc, tc, ctx

    def pst(self, shape):
        """Shared small PSUM scratch (single tag, <=512 f32 per partition)."""
        return self.pps.tile(shape, F32, tag="ps", name="ps")

    def load_consts(self, dd):
        nc = self.nc
        pool = self.ctx.enter_context(self.tc.tile_pool(name="consts", bufs=1))
        sb = {}
        for nm in ("WT_q1", "WT_v1", "WT_o1", "WT_q2", "WT_v2", "WT_o2",
                   "WT_k3", "WT_v3", "WT_o3", "WlvT8"):
            t = pool.tile([128, 2, D], F16, tag=nm)
            nc.sync.dma_start(
                out=t[:],
                in_=dd[nm][:].rearrange("(a p) d -> p a d", p=128))
            sb[nm] = t
        for nm in ("q3blk", "kkT"):
            t = pool.tile([128, 2, H], F16, tag=nm)
            nc.sync.dma_start(
                out=t[:], in_=dd[nm][:].rearrange("(a p) h -> p a h", p=128))
            sb[nm] = t
        for nm in ("bq1_c", "bq2_c", "bk3_c", "REVPOS", "M0", "M3",
                   "IDF", "IDB", "IDH", "bo1_r", "bo2_r", "bo3_r", "blv8r"):
            src = dd[nm]
            t = pool.tile(list(src.shape), src.dtype, tag=nm)
            nc.sync.dma_start(out=t[:], in_=src[:])
            sb[nm] = t
        for nm in ("bv1_r", "bv2_r", "bv3_r", "know_r", "gam2_1", "gam2_2",
                   "gam2_3"):
            src = dd[nm]
            n = src.shape[1]
            t = pool.tile([128, n], F32, tag=nm)
            nc.sync.dma_start(
                out=t[:],
                in_=bass.AP(tensor=src, offset=0, ap=[[0, 128], [1, n]]))
            sb[nm] = t
        t = pool.tile([128, H * D], F16, tag="blv8rep")
        nc.sync.dma_start(
            out=t[:],
            in_=bass.AP(tensor=dd["blv8r"], offset=0,
                        ap=[[0, 128], [1, H * D]]))
        sb["blv8rep"] = t
        ones = pool.tile([1, 128], BF16, tag="ones")
        nc.vector.memset(ones[:], 1.0)
        sb["ones"] = ones
        onesh = pool.tile([128, 128], F16, tag="onesh")
        nc.vector.memset(onesh[:], 1.0)
        sb["ONESH"] = onesh
        onesf = pool.tile([1, 128], F32, tag="onesf")
        nc.vector.memset(onesf[:], 1.0)
        sb["onesf"] = onesf
        epst = pool.tile([128, 1], F32, tag="eps")
        nc.vector.memset(epst[:], EPS)
        sb["eps"] = epst
        self.sb = sb
        # pre-touch identities on PE so later transposes carry a single
        # DMA-queue wait (walrus allows only one sync wait on LDWEIGHTS)
        junk = pool.tile([128, 2], F32, tag="junk")
        wf = self.pps.tile([128, 128], F32, tag="ps", name="warmf")
        nc.tensor.transpose(wf[:], sb["IDF"][:], sb["IDF"][:])
        nc.scalar.copy(out=junk[:, 0:1], in_=wf[:, 0:1])
        wb = self.pps.tile([128, 128], BF16, tag="ps", name="warmb")
        nc.tensor.transpose(wb[:], sb["IDB"][:], sb["IDB"][:])
        nc.scalar.copy(out=junk[:, 1:2], in_=wb[:, 0:1])

    def hslice(self, T, h, cols):
        """Head-rows slice of a [128, 2, SEQ] transposed tensor: [32, len]."""
        return _opt(T[(h % 4) * DK:(h % 4 + 1) * DK, h // 4, cols])

    def load_nat(self, dram, pool, tag):
        tiles = []
        for st in range(NQT):
            t = pool.tile([128, D], F16, tag=f"{tag}{st}")
            self.nc.sync.dma_start(out=t[:],
                                   in_=dram[st * 128:(st + 1) * 128, :])
            tiles.append(t)
        return tiles

    def transpose_nat(self, x_tiles, pool, tag, dt=F16):
        """natural f16 [8][128,256] (tiles or APs) -> [128, 2, 1024] f16."""
        nc = self.nc
        xT = pool.tile([128, 2, SEQ], dt, tag=tag)
        for st in range(NQT):
            ps = self.pps.tile([128, 2, 128], F16, tag="ps", name="tp")
            for dh in range(2):
                nc.tensor.transpose(_opt(ps[:, dh, :]),
                                    _opt(x_tiles[st][:, dh * 128:(dh + 1) * 128]),
                                    self.sb["IDH"][:])
            nc.scalar.copy(out=_opt(xT[:, :, st * 128:(st + 1) * 128]),
                           in_=ps[:])
        return xT

    def proj_T(self, xT, wname, bname, pool, tag):
        """out[do, s] = W @ x.T + b : [128, 2, 1024] bf16."""
        nc = self.nc
        W = self.sb[wname]
        out = pool.tile([128, 2, SEQ], F16, tag=tag)
        for dh in range(2):
            for sc in range(2):
                ps = self.pst([128, 512])
                for ih in range(2):
                    nc.tensor.matmul(
                        ps[:], _opt(W[:, ih, dh * 128:(dh + 1) * 128]),
                        _opt(xT[:, ih, sc * 512:(sc + 1) * 512]),
                        start=(ih == 0), stop=(ih == 1))
                nc.scalar.activation(
                    out=_opt(out[:, dh, sc * 512:(sc + 1) * 512]), in_=ps[:],
                    func=AF.Identity, bias=self.sb[bname][:, dh:dh + 1],
                    scale=1.0)
        return out

    def proj_V(self, xT, wname, bname, pool, tag):
        """V natural with ones column: [8][128, H, 33] bf16."""
        nc = self.nc
        W = self.sb[wname]
        bias = self.sb[bname]
        tiles = []
        for st in range(NQT):
            ps = self.pst([128, D])
            for ih in range(2):
                nc.tensor.matmul(ps[:],
                                 _opt(xT[:, ih, st * 128:(st + 1) * 128]),
                                 _opt(W[:, ih, :]),
                                 start=(ih == 0), stop=(ih == 1))
            v = pool.tile([128, H, DK + 1], BF16, tag=f"{tag}{st}")
            nc.vector.tensor_tensor(
                out=v[:, :, 0:DK],
                in0=ps[:].rearrange("p (h d) -> p h d", h=H),
                in1=bias[:].rearrange("p (h d) -> p h d", h=H), op=OP.add)
            nc.vector.memset(_opt(v[:, :, DK:DK + 1]), 1.0)
            tiles.append(v)
        return tiles

    def make_attn_pools(self, lay, actx, attn_pool, psum, bufs=1, zs_bufs=1):
        tc = self.tc
        P = {"attn": [], "attn_pool": attn_pool}
        P["zq"], P["wtq"], P["oq"] = psum
        P["zsp"] = actx.enter_context(tc.tile_pool(name=f"zsp{lay}",
                                                   bufs=zs_bufs))
        P["ep"] = actx.enter_context(tc.tile_pool(name=f"ep{lay}", bufs=bufs))
        P["sp"] = actx.enter_context(tc.tile_pool(name=f"sp{lay}", bufs=bufs))
        P["sml"] = actx.enter_context(
            tc.tile_pool(name=f"sml{lay}", bufs=2 if lay == 3 else 4))
        P["wts"] = actx.enter_context(
            tc.tile_pool(name=f"wts{lay}", bufs=2 if lay == 3 else 3))
        return P

    # ------------------------------------------------ attention q-tile stages
    def attn_stages(self, lay, qt, QT, V, P, c3g=None, ncg=None,
                    cum3pad=None):
        """Return a list of 10 stage closures for one attention q-tile.
        Interleaving the stage lists of two independent streams keeps every
        engine fed and pairs same-function ACT calls (fewer table loads)."""
        nc, sb = self.nc, self.sb
        strict = (lay == 3)
        Kt = (qt + 1) * 128
        rp_ap = _opt(sb["REVPOS"][:, (7 - qt) * 128:(7 - qt) * 128 + Kt])
        at = P["attn_pool"].tile([128, H, DK], F16, tag=f"attn{lay}_{qt}")
        ot = P["oq"].tile([128, H, DK + 1], F32, tag="o", name="o")
        zs = P["zsp"].tile([128, H, Kt], F16, tag="zs", name="zs")
        e = P["ep"].tile([128, H, Kt], BF16, tag="e", name="e")
        S = P["sp"].tile([128, H, Kt], BF16, tag="S", name="S")
        st_ = {}

        if not strict:
            def s0():  # scores -> PSUM -> zs (+fused causal mask)
                for h in range(H):
                    lhs = self.hslice(QT, h, slice(qt * 128, qt * 128 + 128))
                    tp = ((h % 4) * DK, 0)
                    nchunk = (Kt + 511) // 512
                    for ci in range(nchunk):
                        kc = ci * 512
                        cl = min(512, Kt - kc)
                        z = P["zq"].tile([128, cl], F32, tag="z", name="z")
                        nc.tensor.matmul(
                            z[:], lhs,
                            self.hslice(QT, h, slice(kc, kc + cl)),
                            start=True, stop=True,
                            tile_position=tp, skip_group_check=True)
                        if qt == 0:
                            nc.vector.scalar_tensor_tensor(
                                out=_opt(zs[:, h, :]), in0=z[:], scalar=1.0,
                                in1=sb["M0"][:], op0=OP.mult, op1=OP.add)
                        elif h % 2 == 0:
                            nc.vector.tensor_copy(
                                out=_opt(zs[:, h, kc:kc + cl]), in_=z[:])
                        else:
                            nc.scalar.copy(
                                out=_opt(zs[:, h, kc:kc + cl]), in_=z[:])
                if qt > 0:
                    dg = _opt(zs[:, :, Kt - 128:Kt])
                    nc.vector.scalar_tensor_tensor(
                        out=dg, in0=dg, scalar=1.0,
                        in1=_bcmid(sb["M0"][:], H), op0=OP.mult, op1=OP.add)

            def s1():  # e = exp(zs)
                for hg in range(NG):
                    hs = hg * HG
                    nc.scalar.activation(out=_opt(e[:, hs:hs + HG, :]),
                                         in_=_opt(zs[:, hs:hs + HG, :]),
                                         func=AF.Exp)

            def s2():  # reversed scans -> S; E; rEg
                for h in range(H):
                    nc.vector.tensor_tensor_scan(
                        out=_rev(S[:, h, 0:Kt - 1]),
                        data0=_rev(e[:, h, 1:Kt]), data1=_rev(e[:, h, 1:Kt]),
                        initial=0.0, op0=OP.add, op1=OP.bypass)
                    nc.vector.memset(_opt(S[:, h, Kt - 1:Kt]), 0.0)
                E = P["sml"].tile([128, H], F32, tag="E", name="E")
                rEg = P["sml"].tile([128, H], F32, tag="rEg", name="rEg")
                nc.vector.tensor_tensor(out=E[:], in0=_opt(S[:, :, 0:1]),
                                        in1=_opt(e[:, :, 0:1]), op=OP.add)
                nc.vector.reciprocal(out=rEg[:], in_=E[:])
                nc.vector.tensor_tensor(out=rEg[:], in0=rEg[:],
                                        in1=sb[f"gam2_{lay}"][:], op=OP.mult)
                st_["rEg"] = rEg

            def s3():  # u = (S * rEg_h) * pos  (in place on S)
                rEg = st_["rEg"]
                for h in range(H):
                    nc.vector.scalar_tensor_tensor(
                        out=_opt(S[:, h, :]), in0=_opt(S[:, h, :]),
                        scalar=rEg[:, h:h + 1], in1=rp_ap,
                        op0=OP.mult, op1=OP.mult)

            def s4():  # d = sqrt(u)
                for hg in range(NG):
                    hs = hg * HG
                    nc.scalar.activation(out=_opt(S[:, hs:hs + HG, :]),
                                         in_=_opt(S[:, hs:hs + HG, :]),
                                         func=AF.Sqrt)

            def s5():  # eff = exp(-d)
                for hg in range(NG):
                    hs = hg * HG
                    nc.scalar.activation(out=_opt(S[:, hs:hs + HG, :]),
                                         in_=_opt(S[:, hs:hs + HG, :]),
                                         func=AF.Exp, scale=-1.0)

            def s6():  # t = eff * zs  (in place on zs; 1e-5 clamp dropped)
                for hg in range(NG):
                    hs = hg * HG
                    nc.vector.tensor_tensor(out=_opt(zs[:, hs:hs + HG, :]),
                                            in0=_opt(S[:, hs:hs + HG, :]),
                                            in1=_opt(zs[:, hs:hs + HG, :]),
                                            op=OP.mult)

            def s7():  # w = exp(t)  (into e's tile)
                for hg in range(NG):
                    hs = hg * HG
                    nc.scalar.activation(out=_opt(e[:, hs:hs + HG, :]),
                                         in_=_opt(zs[:, hs:hs + HG, :]),
                                         func=AF.Exp)
        else:
            def s0():  # E3/rE3; u = (E3 - cum3) * pos (into S); diag clamp
                e3ps = self.pst([128, 8])
                nc.tensor.transpose(
                    e3ps[:], _opt(cum3pad[:, qt * 128:qt * 128 + 128]),
                    _opt(sb["IDF"][0:8, 0:8]))
                E3 = P["sml"].tile([128, H], F32, tag="E3", name="E3")
                nc.vector.tensor_scalar_max(out=E3[:], in0=e3ps[:],
                                            scalar1=1e-30)
                rE3 = P["sml"].tile([128, H], F32, tag="rE3", name="rE3")
                nc.vector.reciprocal(out=rE3[:], in_=E3[:])
                nc.vector.tensor_tensor(out=rE3[:], in0=rE3[:],
                                        in1=sb["gam2_3"][:], op=OP.mult)
                st_["rE3"] = rE3
                for h in range(H):
                    nc.vector.scalar_tensor_tensor(
                        out=_opt(S[:, h, :]), in0=_opt(ncg[:, h, 0:Kt]),
                        scalar=_opt(E3[:, h:h + 1]), in1=rp_ap,
                        op0=OP.add, op1=OP.mult)
                nc.vector.tensor_scalar_max(
                    out=_opt(S[:, :, Kt - 128:Kt]),
                    in0=_opt(S[:, :, Kt - 128:Kt]), scalar1=0.0)

            def s1():
                pass

            def s2():
                pass

            def s3():
                pass

            def s4():  # d = sqrt(u * gamma^2/E3) per head (scale AP)
                rE3 = st_["rE3"]
                for h in range(H):
                    nc.scalar.activation(
                        out=_opt(S[:, h, :]), in_=_opt(S[:, h, :]),
                        func=AF.Sqrt, scale=_opt(rE3[:, h:h + 1]))

            def s5():  # eff = exp(-d)
                for hg in range(NG):
                    hs = hg * HG
                    nc.scalar.activation(out=_opt(S[:, hs:hs + HG, :]),
                                         in_=_opt(S[:, hs:hs + HG, :]),
                                         func=AF.Exp, scale=-1.0)

            def s6():  # t = eff * c3 (masked diag) into zs
                if qt > 0:
                    nc.vector.tensor_tensor(
                        out=_opt(zs[:, :, 0:Kt - 128]),
                        in0=_opt(S[:, :, 0:Kt - 128]),
                        in1=_opt(c3g[:, :, 0:Kt - 128]), op=OP.mult)
                c3m = P["sml"].tile([128, H, 128], BF16, tag="c3m",
                                    name="c3m")
                nc.vector.tensor_tensor(
                    out=c3m[:], in0=_opt(c3g[:, :, Kt - 128:Kt]),
                    in1=_bcmid(sb["M3"][:], H), op=OP.add)
                nc.vector.tensor_tensor(
                    out=_opt(zs[:, :, Kt - 128:Kt]),
                    in0=_opt(S[:, :, Kt - 128:Kt]), in1=c3m[:], op=OP.mult)

            def s7():  # w = exp(t)  (into e's tile)
                for hg in range(NG):
                    hs = hg * HG
                    nc.scalar.activation(out=_opt(e[:, hs:hs + HG, :]),
                                         in_=_opt(zs[:, hs:hs + HG, :]),
                                         func=AF.Exp)

        def s8():  # transpose w via PE; A @ [V | 1] accumulation
            for h in range(H):
                nblk = qt + 1
                for g0 in range(0, nblk, 4):
                    gl = min(4, nblk - g0)
                    wt_ps = P["wtq"].tile([128, 512], BF16, tag="wt",
                                          name="wt")
                    for j in range(gl):
                        kb = g0 + j
                        nc.tensor.transpose(
                            _opt(wt_ps[:, j * 128:(j + 1) * 128]),
                            _opt(e[:, h, kb * 128:(kb + 1) * 128]),
                            sb["IDB"][:])
                    wt_sb = P["wts"].tile([128, 512], BF16, tag="wts",
                                          name="wts")
                    if (h + g0 // 4) % 2 == 0:
                        nc.vector.tensor_copy(
                            out=_opt(wt_sb[:, 0:gl * 128]),
                            in_=_opt(wt_ps[:, 0:gl * 128]))
                    else:
                        nc.scalar.copy(
                            out=_opt(wt_sb[:, 0:gl * 128]),
                            in_=_opt(wt_ps[:, 0:gl * 128]))
                    for j in range(gl):
                        kb = g0 + j
                        nc.tensor.matmul(
                            _opt(ot[:, h, :]),
                            _opt(wt_sb[:, j * 128:(j + 1) * 128]),
                            _opt(V[kb][:, h, :]),
                            start=(kb == 0), stop=(kb == qt),
                            skip_group_check=True)

        def s9():  # normalize attn = o / W
            Wg = P["sml"].tile([128, H], F32, tag="Wg", name="Wg")
            rW = P["sml"].tile([128, H], F32, tag="rW", name="rW")
            nc.vector.tensor_scalar_max(
                out=Wg[:], in0=_opt(ot[:, :, DK:DK + 1]), scalar1=1e-30)
            nc.vector.reciprocal(out=rW[:], in_=Wg[:])
            nc.vector.tensor_tensor(
                out=at[:], in0=_opt(ot[:, :, 0:DK]), in1=_bc(rW[:], DK),
                op=OP.mult)
            P["attn"].append(at)

        return [s0, s1, s2, s3, s4, s5, s6, s7, s8, s9]

    # ------------------------------------------------ out proj + LN
    def out_ln(self, lay, attn, res_tiles, hpool, spool, tpool):
        """Out-projection + residual + LN (affine folded downstream).
        Batches the LN sqrt calls to minimize ACT table flips."""
        nc, sb = self.nc, self.sb
        attnT = self.transpose_nat(
            [a[:].rearrange("p h d -> p (h d)") for a in attn],
            tpool, tag=f"attnT{lay}")
        W = sb[f"WT_o{lay}"]
        xs, mvs = [], []
        for st in range(NQT):
            ps = self.pst([128, D])
            for ih in range(2):
                nc.tensor.matmul(ps[:],
                                 _opt(attnT[:, ih, st * 128:(st + 1) * 128]),
                                 _opt(W[:, ih, :]), start=(ih == 0), stop=False)
            nc.tensor.matmul(ps[:], sb["ones"][:], sb[f"bo{lay}_r"][:],
                             start=False, stop=True)
            res = res_tiles[st] if isinstance(res_tiles, list) else res_tiles
            x = tpool.tile([128, D], F16, tag=f"lnx{lay}_{st}", name="lnx")
            nc.vector.tensor_tensor(out=x[:], in0=ps[:], in1=res[:], op=OP.add)
            stats = spool.tile([128, 6], F32, tag="bnst", name="bnst")
            mv = tpool.tile([128, 2], F32, tag=f"bnmv{lay}_{st}", name="bnmv")
            nc.vector.bn_stats(out=stats[:], in_=x[:])
            nc.vector.bn_aggr(out=mv[:], in_=stats[:])
            xs.append(x)
            mvs.append(mv)
        sds = spool.tile([128, NQT], F32, tag="sds", name="sds")
        for st in range(NQT):
            nc.scalar.activation(out=_opt(sds[:, st:st + 1]),
                                 in_=_opt(mvs[st][:, 1:2]), func=AF.Sqrt,
                                 bias=sb["eps"][:], scale=1.0)
        rstds = spool.tile([128, NQT], F32, tag="rstds", name="rstds")
        nc.vector.reciprocal(out=rstds[:], in_=sds[:])
        out_tiles = []
        for st in range(NQT):
            ho = hpool.tile([128, D], F16, tag=f"h{lay}_{st}")
            nc.vector.tensor_scalar(
                out=ho[:], in0=xs[st][:], scalar1=_opt(mvs[st][:, 0:1]),
                scalar2=_opt(rstds[:, st:st + 1]),
                op0=OP.subtract, op1=OP.mult)
            out_tiles.append(ho)
        return out_tiles

    # ------------------------------------------------ layer-3 prologue
    def l3_rows(self, h1T, mpool, lpool):
        nc, sb = self.nc, self.sb
        KT3 = self.proj_T(h1T, "WT_k3", "bk3_c", mpool, tag="KT3")
        c3 = mpool.tile([8, SEQ], BF16, tag="c3")
        c3f = mpool.tile([8, SEQ], F32, tag="c3f")
        for sc in range(2):
            ps = self.pst([8, 512])
            for ih in range(2):
                nc.tensor.matmul(ps[:], _opt(sb["q3blk"][:, ih, :]),
                                 _opt(KT3[:, ih, sc * 512:(sc + 1) * 512]),
                                 start=(ih == 0), stop=(ih == 1))
            nc.vector.tensor_copy(out=_opt(c3[:, sc * 512:(sc + 1) * 512]),
                                  in_=ps[:])
            nc.scalar.copy(out=_opt(c3f[:, sc * 512:(sc + 1) * 512]),
                           in_=ps[:])
        e3 = mpool.tile([8, SEQ], F32, tag="e3")
        nc.scalar.activation(out=e3[:], in_=c3f[:], func=AF.Exp)
        cum3pad = lpool.tile([8, SEQ + 128], F32, tag="cum3pad")
        nc.vector.memset(_opt(cum3pad[:, 0:1]), 0.0)
        nc.vector.tensor_tensor_scan(
            out=_opt(cum3pad[:, 1:SEQ + 1]), data0=e3[:], data1=e3[:],
            initial=0.0, op0=OP.add, op1=OP.bypass)
        nc.vector.memset(_opt(cum3pad[:, SEQ + 1:]), 0.0)
        # stage all 8 head-rows to partition 0 (two DMAs)
        stc = mpool.tile([1, 8, SEQ], BF16, tag="stc", name="stc")
        stn = mpool.tile([1, 8, SEQ], F32, tag="stn", name="stn")
        nc.sync.dma_start(out=stc[:], in_=c3[:])
        nc.sync.dma_start(out=stn[:], in_=cum3pad[:, 1:SEQ + 1])
        c3g = lpool.tile([128, H, SEQ], BF16, tag="c3g")
        ncg = lpool.tile([128, H, SEQ], F32, tag="ncg")
        for h in range(H):
            for sc in range(2):
                ps = self.pst([128, 512])
                nc.tensor.matmul(ps[:], sb["ones"][:],
                                 _opt(stc[:, h, sc * 512:(sc + 1) * 512]),
                                 start=True, stop=True)
                nc.scalar.copy(out=_opt(c3g[:, h, sc * 512:(sc + 1) * 512]),
                               in_=ps[:])
                ps2 = self.pst([128, 512])
                nc.tensor.matmul(ps2[:], sb["onesf"][:],
                                 _opt(stn[:, h, sc * 512:(sc + 1) * 512]),
                                 start=True, stop=True)
                nc.vector.tensor_scalar_mul(
                    out=_opt(ncg[:, h, sc * 512:(sc + 1) * 512]), in0=ps2[:],
                    scalar1=-1.0)
        return c3g, ncg, cum3pad

    # ------------------------------------------------ final mixture
    def final_alphas(self, xT_q, tpool, spool):
        """alpha = softmax(kk @ q_emb) -- depends only on x_q; emitted early."""
        nc, sb = self.nc, self.sb
        als = []
        for st in range(NQT):
            bps = self.pst([128, H])
            for ih in range(2):
                nc.tensor.matmul(bps[:],
                                 _opt(xT_q[:, ih, st * 128:(st + 1) * 128]),
                                 _opt(sb["kkT"][:, ih, :]),
                                 start=(ih == 0), stop=(ih == 1))
            nmax = spool.tile([128, 1], F32, tag="nmax", name="nmax")
            nc.vector.tensor_reduce(out=nmax[:], in_=bps[:],
                                    axis=mybir.AxisListType.X, op=OP.max,
                                    negate=True)
            au = spool.tile([128, H], F32, tag="au", name="au")
            sa = spool.tile([128, 1], F32, tag="sa", name="sa")
            nc.scalar.activation(out=au[:], in_=bps[:], func=AF.Exp,
                                 bias=nmax[:], scale=1.0, accum_out=sa[:])
            rsa = spool.tile([128, 1], F32, tag="rsa", name="rsa")
            nc.vector.reciprocal(out=rsa[:], in_=sa[:])
            al = tpool.tile([128, H], F32, tag=f"al{st}", name="al")
            nc.vector.tensor_scalar_mul(out=al[:], in0=au[:], scalar1=rsa[:])
            als.append(al)
        return als

    def final(self, hh, als, out_dram, spool, tpool, vq):
        nc, sb = self.nc, self.sb
        hhT = self.transpose_nat(hh, tpool, tag="hhT")
        for st in range(NQT):
            al = als[st]
            acc = spool.tile([128, D], F32, tag="facc", name="facc")
            for h in range(H):
                o = (h % 4) * DK
                vps = vq.tile([128, D], F32, tag="vps", name="vps")
                nc.tensor.matmul(
                    vps[:],
                    self.hslice(hhT, h, slice(st * 128, st * 128 + 128)),
                    _opt(sb["WlvT8"][o:o + DK, h // 4, :]),
                    start=True, stop=False,
                    tile_position=(o, 0), skip_group_check=True)
                nc.tensor.matmul(
                    vps[:], _opt(sb["ONESH"][o:o + 1, :]),
                    _opt(sb["blv8rep"][o:o + 1, h * D:(h + 1) * D]),
                    start=False, stop=True,
                    tile_position=(o, 0), skip_group_check=True)
                vsb = spool.tile([128, D], F16, tag="vsb", name="vsb")
                nc.scalar.activation(out=vsb[:], in_=vps[:], func=AF.Sigmoid)
                if h == 0:
                    nc.vector.tensor_scalar_mul(
                        out=acc[:], in0=vsb[:], scalar1=al[:, 0:1])
                else:
                    nc.vector.scalar_tensor_tensor(
                        out=acc[:], in0=vsb[:], scalar=al[:, h:h + 1],
                        in1=acc[:], op0=OP.mult, op1=OP.add)
            nc.sync.dma_start(out=out_dram[st * 128:(st + 1) * 128, :],
                              in_=acc[:])


def build(derived, debug=False, stop_after=None):
    nc = bacc.Bacc(None, target_bir_lowering=False)
    dd = {}
    for name, arr in derived.items():
        dt = {np.dtype(np.float32): F32, np.dtype(bf16): BF16,
              np.dtype(np.float16): F16}[np.dtype(arr.dtype)]
        dd[name] = nc.dram_tensor(name, list(arr.shape), dt,
                                  kind="ExternalInput")
    x_q = nc.dram_tensor("x_q", [SEQ, D], F16, kind="ExternalInput")
    x_s = nc.dram_tensor("x_s", [SEQ, D], F16, kind="ExternalInput")
    out = nc.dram_tensor("out", [SEQ, D], F32, kind="ExternalOutput")

    def dump(tiles, name):
        if not debug:
            return
        t = nc.dram_tensor(name, [SEQ, D], F16, kind="ExternalOutput")
        for st in range(NQT):
            ap = tiles[st][:]
            if len(ap.shape) == 3:
                ap = ap.rearrange("p h d -> p (h d)")
            nc.sync.dma_start(out=t[st * 128:(st + 1) * 128, :], in_=ap)

    with tile.TileContext(nc) as tc, contextlib.ExitStack() as ctx:
        kb = KB(nc, tc, ctx)
        kb.pps = ctx.enter_context(
            tc.tile_pool(name="pps", bufs=2, space="PSUM"))
        kb.load_consts(dd)
        sb = kb.sb
        glob = ctx.enter_context(tc.tile_pool(name="glob", bufs=1))
        hpool = ctx.enter_context(tc.tile_pool(name="hpool", bufs=1))

        h1 = h2 = None
        # ---------------- layers 1+2 interleaved ----------------
        with tc.tile_pool(name="rx", bufs=1) as rx, \
                tc.tile_pool(name="rs", bufs=2) as rs, \
                contextlib.ExitStack() as actx:
            xq_nat = kb.load_nat(x_q, rx, "xq")
            xs_nat = kb.load_nat(x_s, rx, "xs")
            xT_q = kb.transpose_nat(xq_nat, glob, tag="xTq")
            xT_s = kb.transpose_nat(xs_nat, rx, tag="xTs")
            QT1 = kb.proj_T(xT_q, "WT_q1", "bq1_c", rx, tag="QT1")
            QT2 = kb.proj_T(xT_s, "WT_q2", "bq2_c", rx, tag="QT2")
            V1 = kb.proj_V(xT_q, "WT_v1", "bv1_r", rx, tag="V1")
            V2 = kb.proj_V(xT_s, "WT_v2", "bv2_r", rx, tag="V2")
            als = kb.final_alphas(xT_q, glob, rs)
            if stop_after in (None, "l1", "l2", "pro", "l3"):
                zq = actx.enter_context(
                    tc.tile_pool(name="zq", bufs=2, space="PSUM"))
                wtq = actx.enter_context(
                    tc.tile_pool(name="wtq", bufs=2, space="PSUM"))
                oq = actx.enter_context(
                    tc.tile_pool(name="oq", bufs=2, space="PSUM"))
                psum = (zq, wtq, oq)
                P1 = kb.make_attn_pools(1, actx, rx, psum)
                P2 = kb.make_attn_pools(2, actx, rx, psum)
                for qt in range(NQT):
                    a = kb.attn_stages(1, qt, QT1, V1, P1)
                    b = kb.attn_stages(2, qt, QT2, V2, P2)
                    for f1, f2 in zip(a, b):
                        f1()
                        f2()
                attn1, attn2 = P1["attn"], P2["attn"]
                dump(attn1, "dbg_attn1")
                h1 = kb.out_ln(1, attn1, xq_nat, hpool, rs, rx)
                h2 = kb.out_ln(2, attn2, xs_nat, hpool, rs, rx)

        if stop_after in (None, "pro", "l3"):
            dump(h1, "dbg_h1")
            dump(h2, "dbg_h2")
        # ---------------- layer 3 prologue ----------------
        if stop_after in (None, "pro", "l3"):
            lpool = ctx.enter_context(tc.tile_pool(name="l3pool", bufs=1))
            with tc.tile_pool(name="l3tmp", bufs=1) as l3tmp:
                h1T = kb.transpose_nat(h1, l3tmp, tag="h1T")
                h2T = kb.transpose_nat(h2, l3tmp, tag="h2T")
                V3 = kb.proj_V(h2T, "WT_v3", "bv3_r", lpool, tag="V3")
                c3g, ncg, cum3pad = kb.l3_rows(h1T, l3tmp, lpool)
                if stop_after == "pro":
                    for st in range(NQT):
                        o32 = l3tmp.tile([128, D], F32, tag=f"o32_{st}")
                        nc.vector.tensor_copy(
                            out=o32[:].rearrange("p (h d) -> p h d", h=H),
                            in_=V3[st][:, :, 0:DK])
                        nc.sync.dma_start(out=out[st * 128:(st + 1) * 128, :],
                                          in_=o32[:])
        # ---------------- layer 3 + final ----------------
        if stop_after in (None, "l3"):
            with tc.tile_pool(name="r3s", bufs=2) as r3s, \
                    tc.tile_pool(name="r3", bufs=1) as r3, \
                    contextlib.ExitStack() as actx3:
                wtq3 = actx3.enter_context(
                    tc.tile_pool(name="wtq3", bufs=2, space="PSUM"))
                oq3 = actx3.enter_context(
                    tc.tile_pool(name="oq3", bufs=2, space="PSUM"))
                P3 = kb.make_attn_pools(3, actx3, r3, (None, wtq3, oq3),
                                        bufs=2, zs_bufs=1)
                for q0 in range(0, NQT, 2):
                    a = kb.attn_stages(3, q0, None, V3, P3, c3g=c3g,
                                       ncg=ncg, cum3pad=cum3pad)
                    b = kb.attn_stages(3, q0 + 1, None, V3, P3, c3g=c3g,
                                       ncg=ncg, cum3pad=cum3pad)
                    for fa, fb in zip(a, b):
                        fa()
                        fb()
                attn3 = P3["attn"]
                dump(attn3, "dbg_attn3")
                hh = kb.out_ln(3, attn3, sb["know_r"], r3, r3s, r3)
                dump(hh, "dbg_hh")
                if stop_after is None:
                    vq = actx3.enter_context(
                        tc.tile_pool(name="vq", bufs=2, space="PSUM"))
                    kb.final(hh, als, out, r3s, r3, vq)
    nc.compile()
    return nc


_CACHE = {}


def kernel(**inputs):
    drv = host_prep(inputs)
    if "nc" not in _CACHE:
        _CACHE["nc"] = build(drv)
    nc = _CACHE["nc"]
    q = np.asarray(inputs["q_emb"], np.float32).astype(np.float16)
    s = np.asarray(inputs["s_emb"], np.float32).astype(np.float16)
    in_maps = []
    for b in range(BS):
        m = dict(drv)
        m["x_q"] = np.ascontiguousarray(q[b])
        m["x_s"] = np.ascontiguousarray(s[b])
        in_maps.append(m)
    from concourse.bass_utils import run_bass_kernel_spmd
    res = run_bass_kernel_spmd(nc, in_maps, core_ids=list(range(BS)))
    out = np.stack([np.asarray(res.results[b]["out"]) for b in range(BS)],
                   axis=0)
    return out.astype(np.float32)


if __name__ == "__main__":
    print("kernel module loaded OK")
